# revision 29
# baseline (speedup 1.0000x reference)
# Trainium2 Bass kernel for nn_CrossAttention (B=1, I=J=1024, C_S=1024,
# C_Z=128, H=16, D=64), sharded over the query dim i across 8 NeuronCores.
#
# Host prep per core (layout + precision only, no math): slice the i-shard,
# pre-transpose every matmul operand into its device compute layout and cast
# to bf16 so each tensor DMAs straight into SBUF with zero PE transposes:
#   sT_c   [CS, NI]    = s_c^T          kinT [CS, J] = k_in^T
#   biasT_c[CZ, J, NI] = bias_c^T       w*T  [CS, CS] = W^T (all five)
#
# Per-core device program (i-slice of 128 query rows):
#   qT[f,i] = (WqT^T sT + bq)/sqrt(D)   kT[f,j] = WkT^T kinT
#   v[j,h,d|mask] = kinT^T WvT          g[i,f] = sigmoid(sT^T WgT)
#   z[i,j,h] = bias^T Wz  -- per-j matmul: lhsT = biasT[:,j,:] (c,i), rhs = wz
#   scores[i,j] = qT_h^T kT_h + z;  softmax over free dim j without max-sub,
#   denominator folded into the attn@v matmul via a mask column in v_aug.
#   out = (g * o) @ Wo^T
#
# kernel(**inputs) takes FULL inputs, shards on host, runs SPMD on cores 0-7,
# gathers to the full [1, 1024, 1024] f32 output.

import numpy as np

B, I, J, CS, CZ, H, D = 1, 1024, 1024, 1024, 128, 16, 64
NCORES = 8
NI = I // NCORES  # 128 query rows per core
P = 128
NZC = 32  # bias/z chunks of 32 j each
ZW = J // NZC  # 32 j per chunk

_last_results = None


def _build_program():
    from contextlib import ExitStack

    import concourse.mybir as mybir
    import concourse.tile as tile
    from concourse import bacc
    from concourse.masks import make_identity

    f32 = mybir.dt.float32
    bf16 = mybir.dt.bfloat16
    f8 = mybir.dt.float8e4
    AF = mybir.ActivationFunctionType
    ALU = mybir.AluOpType

    nc = bacc.Bacc("TRN2", target_bir_lowering=False, debug=False)

    # ---- dram io (pre-transposed, DMA-native [p, co, f] packing on host) ----
    KC = CS // P  # 8 contraction chunks

    sT_d = nc.dram_tensor("sT_c", [P, KC, NI], bf16, kind="ExternalInput").ap()
    biasT_d = nc.dram_tensor("biasT_c", [CZ, J, NI], f8, kind="ExternalInput").ap()
    kinT_d = nc.dram_tensor("kinT", [P, KC, J], bf16, kind="ExternalInput").ap()
    mask_d = nc.dram_tensor("mask", [J], f32, kind="ExternalInput").ap()
    wqT_d = nc.dram_tensor("wqT", [P, KC, CS], bf16, kind="ExternalInput").ap()
    wkT_d = nc.dram_tensor("wkT", [P, KC, CS], bf16, kind="ExternalInput").ap()
    wvT_d = nc.dram_tensor("wvT", [P, KC, CS], bf16, kind="ExternalInput").ap()
    wgT_d = nc.dram_tensor("wgT", [P, KC, CS], bf16, kind="ExternalInput").ap()
    woT_d = nc.dram_tensor("woT", [P, KC, CS], bf16, kind="ExternalInput").ap()
    bq_d = nc.dram_tensor("b_q", [CS], f32, kind="ExternalInput").ap()
    wz_d = nc.dram_tensor("w_z", [CZ, H], f32, kind="ExternalInput").ap()
    out_d = nc.dram_tensor("out", [NI, CS], f32, kind="ExternalOutput").ap()

    with tile.TileContext(nc) as tc, ExitStack() as ctx:
        pool = lambda name, bufs: ctx.enter_context(tc.tile_pool(name=name, bufs=bufs))
        ppool = lambda name, bufs: ctx.enter_context(
            tc.tile_pool(name=name, bufs=bufs, space="PSUM")
        )

        const = pool("const", 1)
        wt_p = pool("wt", 3)  # streamed weight tiles, 16KB/part each
        act_p = pool("act", 1)  # sT, kinT, qT, kT, v, g, z, o
        bstage_p = pool("bstage", 6)  # fp8 bias chunks, 4KB/part each
        st_p = pool("st", 2)
        et_p = pool("et", 2)
        r_p = pool("r", 2)
        outs_p = pool("outs", 1)

        zps = ppool("zps", 2)  # z accumulation [128,512] f32
        pps = ppool("pps", 2)  # projection accumulators [128,512] f32
        qkps = ppool("qkps", 2)
        tps = ppool("tps", 1)  # transpose target (et, goT)
        ops = ppool("ops", 1)  # attn@v accumulator

        def copy_on(eng_is_vector, out, in_):
            if eng_is_vector:
                nc.vector.tensor_copy(out, in_)
            else:
                nc.scalar.copy(out, in_)

        # queue plan: gpsimd = wz cast + bias chunk stream;
        # sync = wq, kinT, wk, wv (fire immediately, wt pool has 3 bufs);
        # scalar = sT/bq/mask consts, then wg, wo at their use sites.
        wz_s = const.tile([CZ, H], bf16)
        nc.gpsimd.dma_start(wz_s, wz_d)  # cast f32 -> bf16 (cast needs gpsimd)

        def load_w(w_ap, tag, eng):
            wT = wt_p.tile([P, KC, CS], bf16, tag="wt", name=tag)
            eng.dma_start(wT, w_ap)
            return wT

        wqT = load_w(wqT_d, "wq", nc.sync)
        kinT = act_p.tile([P, KC, J], bf16, tag="kinT")
        nc.sync.dma_start(kinT, kinT_d)
        wkT = load_w(wkT_d, "wk", nc.sync)
        wvT = load_w(wvT_d, "wv", nc.sync)

        sT = act_p.tile([P, KC, NI], bf16, tag="sT")
        nc.scalar.dma_start(sT, sT_d)
        bq_s = const.tile([P, KC], f32)
        nc.scalar.dma_start(bq_s, bq_d.rearrange("(fo p) -> p fo", p=P))
        mask_s = const.tile([P, KC], f32)
        nc.scalar.dma_start(mask_s, mask_d.rearrange("(jo p) -> p jo", p=P))

        ident = const.tile([P, P], bf16)

        # ---- z: stream bias chunks, per-j matmul against wz ----
        # z_s layout [i_part, chunk, j_in_chunk, h]
        z_s = act_p.tile([P, NZC, ZW, H], bf16, tag="z")

        def emit_z(chunks):
            for m in chunks:
                bt = bstage_p.tile([CZ, ZW, NI], f8, tag="bt", name=f"bt_{m}")
                nc.gpsimd.dma_start(bt, biasT_d[:, m * ZW : (m + 1) * ZW, :])
                zp = zps.tile([P, ZW * H], f32, tag="zp", name=f"zp_{m}")
                for jw in range(ZW):
                    nc.tensor.matmul(
                        zp[:, jw * H : (jw + 1) * H],
                        bt[:, jw, :],
                        wz_s,
                        start=True,
                        stop=True,
                    )
                copy_on(m % 2 == 0, z_s[:, m], zp)

        emit_z(range(0, 6))

        # ---- q projection: qT [f, i] = WqT^T sT, scaled, +bq ----
        qT = act_p.tile([P, KC, NI], bf16, tag="qT")
        for fo in range(KC):
            ps = pps.tile([P, 512], f32, tag="pp", name=f"qp_{fo}")
            for co in range(KC):
                nc.tensor.matmul(
                    ps[:, :NI],
                    wqT[:, co, fo * P : (fo + 1) * P],
                    sT[:, co, :],
                    start=(co == 0),
                    stop=(co == KC - 1),
                )
            nc.vector.tensor_scalar(
                qT[:, fo, :],
                ps[:, :NI],
                bq_s[:, fo : fo + 1],
                1.0 / np.sqrt(D),
                ALU.add,
                ALU.mult,
            )

        emit_z(range(6, 9))

        # ---- k projection: kT [f, j] = WkT^T kinT ----
        kT = act_p.tile([P, KC, J], bf16, tag="kT")
        for fo in range(KC):
            for jh in range(2):
                ps = pps.tile([P, 512], f32, tag="pp", name=f"kp_{fo}_{jh}")
                for co in range(KC):
                    nc.tensor.matmul(
                        ps,
                        wkT[:, co, fo * P : (fo + 1) * P],
                        kinT[:, co, jh * 512 : (jh + 1) * 512],
                        start=(co == 0),
                        stop=(co == KC - 1),
                    )
                copy_on(jh == 0, kT[:, fo, jh * 512 : (jh + 1) * 512], ps)

        emit_z(range(9, 12))

        # ---- v projection: v [j, h, d|mask] = kinT^T WvT, masked ----
        v_s = act_p.tile([P, KC, H, D + 1], bf16, tag="v")
        for jo in range(KC):
            for fh in range(2):
                ps = pps.tile([P, 512], f32, tag="pp", name=f"vp_{jo}_{fh}")
                for co in range(KC):
                    nc.tensor.matmul(
                        ps,
                        kinT[:, co, jo * P : (jo + 1) * P],
                        wvT[:, co, fh * 512 : (fh + 1) * 512],
                        start=(co == 0),
                        stop=(co == KC - 1),
                    )
                nc.vector.tensor_scalar_mul(
                    v_s[:, jo, fh * 8 : (fh + 1) * 8, 0:D],
                    ps,
                    mask_s[:, jo : jo + 1],
                )
            nc.vector.tensor_copy(
                v_s[:, jo, :, D : D + 1],
                mask_s[:, jo : jo + 1, None].to_broadcast((P, H, 1)),
            )

        wgT = load_w(wgT_d, "wg", nc.scalar)
        emit_z(range(12, 16))

        # ---- g projection: g [i, f] = sigmoid(sT^T WgT) ----
        g_s = act_p.tile([P, CS], bf16, tag="g")
        for fh in range(2):
            ps = pps.tile([P, 512], f32, tag="pp", name=f"gp_{fh}")
            for co in range(KC):
                nc.tensor.matmul(
                    ps,
                    sT[:, co, :],
                    wgT[:, co, fh * 512 : (fh + 1) * 512],
                    start=(co == 0),
                    stop=(co == KC - 1),
                )
            nc.scalar.activation(g_s[:, fh * 512 : (fh + 1) * 512], ps, AF.Sigmoid)

        woT = load_w(woT_d, "wo", nc.scalar)
        emit_z(range(16, 20))
        make_identity(nc, ident)

        # ---- attention (scores-major-i), two j-halves ----
        o_s = act_p.tile([P, CS], bf16, tag="o")
        o_acc = act_p.tile([P, H, D + 1], f32, tag="oacc")

        def emit_attn_pass(jh):
            for h in range(H):
                fo, pb = h // 2, (h % 2) * D
                qk = qkps.tile([P, 512], f32, tag="qk", name=f"qk_{jh}_{h}")
                nc.tensor.matmul(
                    qk,
                    qT[pb : pb + D, fo, :],
                    kT[pb : pb + D, fo, jh * 512 : (jh + 1) * 512],
                    start=True,
                    stop=True,
                )
                st = st_p.tile([P, 512], f32, tag="st", name=f"st_{jh}_{h}")
                nc.vector.tensor_tensor(
                    st,
                    qk,
                    z_s[:, 16 * jh : 16 * (jh + 1), :, h].rearrange(
                        "p a b -> p (a b)"
                    ),
                    ALU.add,
                )
                et = et_p.tile([P, 512], bf16, tag="et", name=f"et_{jh}_{h}")
                nc.scalar.activation(et, st, AF.Exp)
                tb = tps.tile([P, 512], bf16, tag="tb", name=f"etb_{jh}_{h}")
                for jl in range(4):
                    nc.tensor.transpose(
                        tb[:, jl * P : (jl + 1) * P],
                        et[:, jl * P : (jl + 1) * P],
                        ident,
                    )
                etT = et_p.tile([P, 4, P], bf16, tag="etT", name=f"etT_{jh}_{h}")
                copy_on(h % 2 == 0, etT, tb)
                op = ops.tile([P, 512], f32, tag="op", name=f"op_{jh}_{h}")
                for jc4 in range(4):
                    nc.tensor.matmul(
                        op[:, : D + 1],
                        etT[:, jc4, :],
                        v_s[:, jh * 4 + jc4, h, :],
                        start=(jc4 == 0),
                        stop=(jc4 == 3),
                    )
                if jh == 0:
                    nc.vector.tensor_copy(o_acc[:, h, :], op[:, : D + 1])
                else:
                    nc.vector.tensor_tensor(
                        o_acc[:, h, :], op[:, : D + 1], o_acc[:, h, :], ALU.add
                    )
                    # normalize + gate this head immediately so the tail
                    # after the last head is short
                    rec = r_p.tile([P, 1], f32, tag="r", name=f"rec_{h}")
                    nc.vector.reciprocal(rec, o_acc[:, h, D : D + 1])
                    nc.vector.tensor_scalar_mul(
                        o_s[:, h * D : (h + 1) * D], o_acc[:, h, 0:D], rec
                    )
                    nc.vector.tensor_mul(
                        g_s[:, h * D : (h + 1) * D],
                        g_s[:, h * D : (h + 1) * D],
                        o_s[:, h * D : (h + 1) * D],
                    )

        emit_attn_pass(0)
        emit_z(range(20, 32))
        emit_attn_pass(1)

        # ---- output projection (g_s now holds g * o) ----
        goT = act_p.tile([P, KC, NI], bf16, tag="goT")
        for gh in range(2):
            tb = tps.tile([P, 512], bf16, tag="tb", name=f"gob_{gh}")
            for fo in range(gh * 4, gh * 4 + 4):
                nc.tensor.transpose(
                    tb[:, (fo % 4) * P : (fo % 4 + 1) * P],
                    g_s[:, fo * P : (fo + 1) * P],
                    ident,
                )
            nc.vector.tensor_copy(goT[:, gh * 4 : (gh + 1) * 4, :], tb)

        for fh in range(2):
            ps = pps.tile([P, 512], f32, tag="pp", name=f"op_{fh}")
            for fo in range(KC):
                nc.tensor.matmul(
                    ps,
                    goT[:, fo, :],
                    woT[:, fo, fh * 512 : (fh + 1) * 512],
                    start=(fo == 0),
                    stop=(fo == KC - 1),
                )
            out_s = outs_p.tile([P, 512], f32, tag="outs", name=f"out_s{fh}")
            nc.vector.tensor_copy(out_s, ps)
            nc.sync.dma_start(out_d[:, fh * 512 : (fh + 1) * 512], out_s)

    nc.compile()
    return nc


def kernel(**inputs):
    global _last_results
    import ml_dtypes
    from concourse.bass_utils import run_bass_kernel_spmd

    bf16 = ml_dtypes.bfloat16
    f8 = ml_dtypes.float8_e4m3

    s = np.asarray(inputs["s"], dtype=np.float32)[0]
    k_in = np.asarray(inputs["k_in"], dtype=np.float32)[0]
    mask = np.ascontiguousarray(np.asarray(inputs["mask"], dtype=np.float32)[0])
    bias = np.asarray(inputs["bias"], dtype=np.float32)[0]
    bq = np.ascontiguousarray(np.asarray(inputs["bq"], dtype=np.float32))
    wz = np.ascontiguousarray(np.asarray(inputs["Wz"], dtype=np.float32))
    mult = int(np.asarray(inputs.get("multiplicity", 1)))
    assert mult == 1, f"multiplicity={mult} not supported (B=1)"

    # host-side layout prep: transpose into device compute layouts, pack to
    # the DMA-native [p, co, f] ordering (16KB contiguous per partition),
    # cast bf16 (fp8 for bias)
    def pack(xT):  # [CS, F] -> [P, KC, F]
        return np.ascontiguousarray(
            xT.reshape(CS // P, P, -1).transpose(1, 0, 2).astype(bf16)
        )

    sT = pack(s.T)  # [P, KC, I]
    kinT = pack(k_in.T)  # [P, KC, J]
    wT = {
        k: pack(np.asarray(inputs[k], dtype=np.float32).T)
        for k in ("Wq", "Wk", "Wv", "Wg", "Wo")
    }

    nc = _build_program()

    in_maps = []
    for c in range(NCORES):
        sl = slice(c * NI, (c + 1) * NI)
        in_maps.append(
            {
                "sT_c": np.ascontiguousarray(sT[:, :, sl]),
                "biasT_c": bias[sl].transpose(2, 1, 0).astype(f8),
                "kinT": kinT,
                "mask": mask,
                "wqT": wT["Wq"],
                "wkT": wT["Wk"],
                "wvT": wT["Wv"],
                "wgT": wT["Wg"],
                "woT": wT["Wo"],
                "b_q": bq,
                "w_z": wz,
            }
        )

    try:
        res = run_bass_kernel_spmd(nc, in_maps, core_ids=list(range(NCORES)))
    except Exception:
        # transient device-unrecoverable errors have been observed on a
        # first attempt; one retry has always succeeded
        import time as _time

        _time.sleep(5.0)
        res = run_bass_kernel_spmd(nc, in_maps, core_ids=list(range(NCORES)))
    _last_results = res
    out = np.concatenate([r["out"] for r in res.results], axis=0)
    return out.reshape(B, I, CS).astype(np.float32)


if __name__ == "__main__":
    rng = np.random.default_rng(0)
    ins = {
        "s": rng.standard_normal((B, I, CS), dtype=np.float32),
        "k_in": rng.standard_normal((B, J, CS), dtype=np.float32),
        "mask": np.ones((B, J), np.float32),
        "bias": rng.standard_normal((B, I, J, CZ), dtype=np.float32),
        "Wq": rng.standard_normal((CS, CS), dtype=np.float32) * 0.02,
        "bq": rng.standard_normal((CS,), dtype=np.float32) * 0.02,
        "Wk": rng.standard_normal((CS, CS), dtype=np.float32) * 0.02,
        "Wv": rng.standard_normal((CS, CS), dtype=np.float32) * 0.02,
        "Wg": rng.standard_normal((CS, CS), dtype=np.float32) * 0.02,
        "Wo": rng.standard_normal((CS, CS), dtype=np.float32) * 0.02,
        "Wz": rng.standard_normal((CZ, H), dtype=np.float32) * 0.02,
        "multiplicity": 1,
    }
    out = kernel(**ins)
    print(out.shape, out.dtype)
